# revision 25
# baseline (speedup 1.0000x reference)
"""ChebConv (K=6) message-passing kernel for 8 Trainium2 NeuronCores.

Math: the reference's GraphNetwork pass multiplies each node's features by a
per-node scalar s = (deg - in_w) / max(deg) (deg = segment_sum(edges, senders),
in_w = segment_sum(edges, receivers)), and the Chebyshev recurrence
Tx_k = 2*Tx_{k-1} - Tx_{k-2} stays rank-1 per node: Tx_k = (1 + k*(s-1)) * x.
Hence
    out = X @ WA + s * (X @ WB) + b_tot
with WA = sum_k (1-k) Wk[k], WB = sum_k k Wk[k], b_tot = sum_k bk[k] + bias.

Sharding: nodes are block-sharded over 8 cores (12500 each, padded to 12544).
Edges are routed on the host (index permutation + zero fill only, no float
arithmetic) to the core owning their sender (for deg) / receiver (for in_w),
laid out as a per-node padded slot matrix so each segment-sum becomes a dense
row reduction on device.

Two launches (an in-kernel AllReduce costs ~60us of comm-subsystem init, far
more than a second NEFF):
  A: edge kernel — per-core deg/in_w row-reductions + per-core max(deg).
  host: m = max of the 8 device-computed partial maxima (selection only).
  B: main kernel — s = (deg-in_w)*recip(m), X@[WA|WB] matmuls (fp32r),
     s-scaled combine + bias, all overlapped with DMA.
"""

import sys

sys.path.insert(0, "/opt/trn_rl_repo")

import numpy as np

import concourse.bacc as bacc
import concourse.bass as bass
import concourse.mybir as mybir
import concourse.tile as tile
from concourse import bass_isa
from concourse.bass_utils import run_bass_kernel_spmd

N_NODES = 100000
F = 128
KCH = 6
NCORES = 8
NPC = N_NODES // NCORES       # 12500 nodes per core
T = (NPC + 127) // 128        # 98 node tiles per core
NPAD = T * 128                # 12544 (rows 12500.. are zero padding)
DSLOT_MIN = 64                # per-node edge-slot padding (>= max degree)

f32 = mybir.dt.float32
f32r = mybir.dt.float32r
bf16 = mybir.dt.bfloat16
fp16 = mybir.dt.float16

# test.py knobs (harness never touches these)
TRACE = False
LAST = {}
MM_MODE = "tfp16"  # "f32" | "f32r" | "tfp16" | "tf32r"

_prog_cache = {}


def _build_edge_program(ds, dr):
    """Launch A: deg = rowsum(pse), inw = rowsum(pre), pmax = max(deg)."""
    nc = bacc.Bacc("TRN2", target_bir_lowering=False, debug=False,
                   num_devices=NCORES)
    A = mybir.AluOpType
    X = mybir.AxisListType.X

    pse_d = nc.dram_tensor("pse", [128, T * ds], fp16, kind="ExternalInput")
    pre_d = nc.dram_tensor("pre", [128, T * dr], fp16, kind="ExternalInput")
    degw_d = nc.dram_tensor("degw", [128, 2 * T], f32, kind="ExternalOutput")
    pmax_d = nc.dram_tensor("pmax", [1, 1], f32, kind="ExternalOutput")

    CH = [(0, 25), (25, 25), (50, 24), (74, 24)]
    with tile.TileContext(nc) as tc:
        with (
            tc.tile_pool(name="edge", bufs=1) as edgep,
            tc.tile_pool(name="small", bufs=1) as smallp,
        ):
            degw = smallp.tile([128, 2 * T], f32)
            pse_sb = edgep.tile([128, T, ds], fp16)
            pre_sb = edgep.tile([128, T, dr], fp16)
            for t0, n in CH:
                nc.sync.dma_start(
                    pse_sb[:, t0 : t0 + n, :],
                    pse_d[:, t0 * ds : (t0 + n) * ds].rearrange("p (t d) -> p t d", d=ds))
                nc.vector.tensor_reduce(degw[:, t0 : t0 + n], pse_sb[:, t0 : t0 + n, :],
                                        axis=X, op=A.add)
            for t0, n in CH:
                nc.sync.dma_start(
                    pre_sb[:, t0 : t0 + n, :],
                    pre_d[:, t0 * dr : (t0 + n) * dr].rearrange("p (t d) -> p t d", d=dr))
                nc.vector.tensor_reduce(degw[:, T + t0 : T + t0 + n], pre_sb[:, t0 : t0 + n, :],
                                        axis=X, op=A.add)

            dmax = smallp.tile([128, 1], f32)
            nc.vector.tensor_reduce(dmax[:, :], degw[:, :T], axis=X, op=A.max)
            pmax = smallp.tile([128, 1], f32)
            nc.gpsimd.partition_all_reduce(pmax[:, :], dmax[:, :], channels=128,
                                           reduce_op=bass_isa.ReduceOp.max)
            nc.sync.dma_start(degw_d[:, :], degw[:, :])
            nc.sync.dma_start(pmax_d[:, :], pmax[0:1, 0:1])

    nc.compile()
    return nc


def _build_main_program(mm_mode):
    """Launch B: out = X@WA + s*(X@WB) + b_tot, s = (deg-inw)*recip(m)."""
    nc = bacc.Bacc("TRN2", target_bir_lowering=False, debug=False,
                   num_devices=NCORES)
    A = mybir.AluOpType
    X = mybir.AxisListType.X
    mmdt = f32r if mm_mode == "f32r" else f32

    xt_d = nc.dram_tensor("xt", [F, NPAD], mmdt, kind="ExternalInput")
    wk_d = nc.dram_tensor("wk", [KCH, F, F], f32, kind="ExternalInput")
    bkb_d = nc.dram_tensor("bkb", [1, (KCH + 1) * F], f32, kind="ExternalInput")
    degw_d = nc.dram_tensor("degw", [128, 2 * T], f32, kind="ExternalInput")
    mmax_d = nc.dram_tensor("mmax", [1, 1], f32, kind="ExternalInput")
    out_d = nc.dram_tensor("out", [NPAD, F], f32, kind="ExternalOutput")

    XCH = 7                    # xt / out DMA chunks
    TCH = T // XCH             # 14 node tiles per chunk
    G = 7                      # node tiles per PSUM group (7KB -> 4 banks)

    with tile.TileContext(nc) as tc:
        with (
            tc.tile_pool(name="const", bufs=1) as constp,
            tc.tile_pool(name="xt", bufs=1) as xtp,
            tc.tile_pool(name="outp", bufs=1) as outp,
            tc.tile_pool(name="small", bufs=1) as smallp,
            tc.tile_pool(name="tmp", bufs=4) as tmpp,
            tc.tile_pool(name="ps", bufs=2, space="PSUM") as psp,
        ):
            # ---- constants (tiny DMAs first so wab is ready early) --------
            wk_sb = constp.tile([128, KCH, F], f32)
            nc.sync.dma_start(wk_sb[:, :, :], wk_d.ap().rearrange("k p f -> p k f"))
            bkb_sb = constp.tile([1, (KCH + 1) * F], f32)
            nc.sync.dma_start(bkb_sb[:, :], bkb_d[:, :])
            degw_sb = smallp.tile([128, 2 * T], f32)
            nc.sync.dma_start(degw_sb[:, :], degw_d[:, :])

            # WA | WB:  WA = W0 - W2 - 2W3 - 3W4 - 4W5,
            #           WB = W1 + 2W2 + 3W3 + 4W4 + 5W5
            wab = constp.tile([128, 2 * F], mmdt)
            wa, wb = wab[:, 0:F], wab[:, F : 2 * F]
            nc.vector.scalar_tensor_tensor(wa, wk_sb[:, 2, :], -1.0, wk_sb[:, 0, :], op0=A.mult, op1=A.add)
            nc.vector.scalar_tensor_tensor(wa, wk_sb[:, 3, :], -2.0, wa, op0=A.mult, op1=A.add)
            nc.vector.scalar_tensor_tensor(wa, wk_sb[:, 4, :], -3.0, wa, op0=A.mult, op1=A.add)
            nc.vector.scalar_tensor_tensor(wa, wk_sb[:, 5, :], -4.0, wa, op0=A.mult, op1=A.add)
            nc.vector.scalar_tensor_tensor(wb, wk_sb[:, 2, :], 2.0, wk_sb[:, 1, :], op0=A.mult, op1=A.add)
            nc.vector.scalar_tensor_tensor(wb, wk_sb[:, 3, :], 3.0, wb, op0=A.mult, op1=A.add)
            nc.vector.scalar_tensor_tensor(wb, wk_sb[:, 4, :], 4.0, wb, op0=A.mult, op1=A.add)
            nc.vector.scalar_tensor_tensor(wb, wk_sb[:, 5, :], 5.0, wb, op0=A.mult, op1=A.add)

            # b_tot broadcast to all partitions via a K=1 outer-product matmul
            btot = smallp.tile([1, F], f32)
            nc.vector.tensor_reduce(
                btot[:, :], bkb_sb.rearrange("p (s f) -> p f s", s=KCH + 1),
                axis=X, op=A.add)
            btot_b = smallp.tile([1, F], bf16)
            nc.vector.tensor_copy(btot_b[:, :], btot[:, :])
            ones_row = smallp.tile([1, F], bf16)
            nc.vector.memset(ones_row[:, :], 1.0)
            btile_ps = psp.tile([128, F], f32, tag="ps")
            nc.tensor.matmul(btile_ps[:, :], ones_row[:, :], btot_b[:, :],
                             start=True, stop=True)
            btile = smallp.tile([128, F], f32)
            nc.vector.tensor_copy(btile[:, :], btile_ps[:, :])

            # ---- s = (deg - inw) * recip(m) -------------------------------
            m_bc = smallp.tile([128, 1], f32)
            map_ = mmax_d[0:1, 0:1]
            nc.sync.dma_start(m_bc[:, :], bass.AP(map_.tensor, map_.offset, [[0, 128], [1, 1]]))
            minv = smallp.tile([128, 1], f32)
            nc.vector.reciprocal(minv[:, :], m_bc[:, :])
            s_sb = smallp.tile([128, T], f32)
            nc.vector.tensor_sub(s_sb[:, :], degw_sb[:, :T], degw_sb[:, T :])
            nc.vector.tensor_scalar_mul(s_sb[:, :], s_sb[:, :], minv[:, 0:1])

            # ---- node features -------------------------------------------
            xt_sb = []
            for c in range(XCH):
                xt_c = xtp.tile([128, TCH * 128], mmdt, name=f"xt{c}")
                nc.sync.dma_start(xt_c[:, :], xt_d[:, c * TCH * 128 : (c + 1) * TCH * 128])
                xt_sb.append(xt_c)

            # ---- matmuls + combine ---------------------------------------
            def bc(ap, reps, width):
                """[128, G] tile slice -> [128, G, width] 0-stride broadcast AP."""
                return bass.AP(ap.tensor, ap.offset, [ap.ap[0], [1, reps], [0, width]])

            for c in range(XCH):
                out_c = outp.tile([128, TCH, F], f32, name=f"out{c}")
                for g in range(TCH // G):
                    t0 = c * TCH + g * G
                    ps = psp.tile([128, G, 2 * F], f32, tag="ps")
                    for u in range(G):
                        j = g * G + u
                        nc.tensor.matmul(ps[:, u, :],
                                         xt_sb[c][:, j * 128 : (j + 1) * 128],
                                         wab[:, :], start=True, stop=True)
                    tmp = tmpp.tile([128, G, F], f32, tag="tmp")
                    g_abs = c * (TCH // G) + g
                    if g_abs % 7 < 5:
                        # scalar engine: per-tile copy with per-partition scale
                        for u in range(G):
                            nc.scalar.activation(tmp[:, u, :], ps[:, u, F : 2 * F],
                                                 mybir.ActivationFunctionType.Copy,
                                                 scale=s_sb[:, t0 + u : t0 + u + 1])
                    else:
                        nc.vector.tensor_tensor(tmp[:, :, :], ps[:, :, F : 2 * F],
                                                bc(s_sb[:, t0 : t0 + G], G, F),
                                                op=A.mult)
                    nc.vector.tensor_tensor(out_c[:, g * G : (g + 1) * G, :],
                                            tmp[:, :, :], ps[:, :, 0:F], op=A.add)
                bt = btile[:, :]
                btile_bc = bass.AP(bt.tensor, bt.offset, [bt.ap[0], [0, TCH], [1, F]])
                nc.vector.tensor_tensor(out_c[:, :, :], out_c[:, :, :], btile_bc, op=A.add)
                nc.sync.dma_start(
                    out_d[c * TCH * 128 : (c + 1) * TCH * 128, :].rearrange("(j p) f -> p j f", p=128),
                    out_c[:, :, :])

    nc.compile()
    return nc


def _build_main_program_t(mm_mode):
    """Launch B (transposed): outT = WA.T@X.T + WB.T@(s*X).T + b_tot, where
    X.T arrives feature-major ([fi, n]) so s varies along the free axis.  The
    s-scaled term is a second accumulating matmul with sx = x*srep; srep is a
    PE outer-product broadcast of s.  Bias rides the ACT evacuation as a
    per-partition bias (fo is the partition axis here).  Host transposes the
    [F, NPAD] result back."""
    nc = bacc.Bacc("TRN2", target_bir_lowering=False, debug=False,
                   num_devices=NCORES)
    A = mybir.AluOpType
    X = mybir.AxisListType.X
    mmdt = fp16 if mm_mode == "tfp16" else f32r

    xt_d = nc.dram_tensor("xt", [F, NPAD], mmdt, kind="ExternalInput")
    wk_d = nc.dram_tensor("wk", [KCH, F, F], f32, kind="ExternalInput")
    bkb_d = nc.dram_tensor("bkb", [1, (KCH + 1) * F], f32, kind="ExternalInput")
    degw_d = nc.dram_tensor("degw", [128, 2 * T], f32, kind="ExternalInput")
    mmax_d = nc.dram_tensor("mmax", [1, 1], f32, kind="ExternalInput")
    odt = fp16 if mm_mode == "tfp16" else f32
    out_d = nc.dram_tensor("out", [F, NPAD], odt, kind="ExternalOutput")

    XCH = 7                    # xt / out DMA chunks (1792 cols each)
    CW = NPAD // XCH           # 1792
    GW = 448                   # matmul group width (PSUM bank = 512 f32 max)
    GPC = CW // GW             # 4 groups per chunk

    from concourse import masks

    with tile.TileContext(nc) as tc:
        with (
            tc.tile_pool(name="const", bufs=1) as constp,
            tc.tile_pool(name="xt", bufs=1) as xtp,
            tc.tile_pool(name="outp", bufs=1) as outp,
            tc.tile_pool(name="small", bufs=1) as smallp,
            tc.tile_pool(name="sx", bufs=3) as sxp,
            tc.tile_pool(name="srepp", bufs=3) as srepp,
            tc.tile_pool(name="psf", bufs=6, space="PSUM") as psf,
            tc.tile_pool(name="pst", bufs=1, space="PSUM") as pst,
            tc.tile_pool(name="dram", bufs=1, space="DRAM") as dramp,
        ):
            # ---- tiny input DMAs (m + degw first: they feed the s chain) --
            with tc.high_priority():
                m_bc = smallp.tile([128, 1], f32)
                map_ = mmax_d[0:1, 0:1]
                nc.sync.dma_start(m_bc[:, :], bass.AP(map_.tensor, map_.offset, [[0, 128], [1, 1]]))
                degw_sb = smallp.tile([128, 2 * T], f32)
                nc.sync.dma_start(degw_sb[:, :], degw_d[:, :])
            wk_sb = constp.tile([128, KCH, F], f32)
            nc.sync.dma_start(wk_sb[:, :, :], wk_d.ap().rearrange("k p f -> p k f"))
            bkb_sb = constp.tile([1, (KCH + 1) * F], f32)
            nc.sync.dma_start(bkb_sb[:, :], bkb_d[:, :])

            # ---- weights: WA | WB in f32, then cast to matmul dtype --------
            wab = constp.tile([128, 2 * F], f32)
            wa, wb = wab[:, 0:F], wab[:, F : 2 * F]
            nc.vector.scalar_tensor_tensor(wa, wk_sb[:, 2, :], -1.0, wk_sb[:, 0, :], op0=A.mult, op1=A.add)
            nc.vector.scalar_tensor_tensor(wa, wk_sb[:, 3, :], -2.0, wa, op0=A.mult, op1=A.add)
            nc.vector.scalar_tensor_tensor(wa, wk_sb[:, 4, :], -3.0, wa, op0=A.mult, op1=A.add)
            nc.vector.scalar_tensor_tensor(wa, wk_sb[:, 5, :], -4.0, wa, op0=A.mult, op1=A.add)
            nc.vector.scalar_tensor_tensor(wb, wk_sb[:, 2, :], 2.0, wk_sb[:, 1, :], op0=A.mult, op1=A.add)
            nc.vector.scalar_tensor_tensor(wb, wk_sb[:, 3, :], 3.0, wb, op0=A.mult, op1=A.add)
            nc.vector.scalar_tensor_tensor(wb, wk_sb[:, 4, :], 4.0, wb, op0=A.mult, op1=A.add)
            nc.vector.scalar_tensor_tensor(wb, wk_sb[:, 5, :], 5.0, wb, op0=A.mult, op1=A.add)
            wa16 = constp.tile([128, F], mmdt)
            wb16 = constp.tile([128, F], mmdt)
            nc.vector.tensor_copy(wa16[:, :], wa)
            nc.vector.tensor_copy(wb16[:, :], wb)

            # ---- s = (deg - inw) * recip(m), transposed to node order ------
            with tc.high_priority():
                minv = smallp.tile([128, 1], f32)
                nc.vector.reciprocal(minv[:, :], m_bc[:, :])
                s_sb = smallp.tile([128, T], f32)
                nc.vector.tensor_sub(s_sb[:, :], degw_sb[:, :T], degw_sb[:, T:])
                nc.vector.tensor_scalar_mul(s_sb[:, :], s_sb[:, :], minv[:, 0:1])
                s16 = smallp.tile([128, 128], fp16)
                nc.vector.memset(s16[:, :], 0.0)
                nc.vector.tensor_copy(s16[:, 0:T], s_sb[:, :])
                ident16 = smallp.tile([128, 128], fp16)
                masks.make_identity(nc, ident16[:, :])
            with tc.high_priority():
                ps_t = pst.tile([128, 128], fp16, tag="pst")
                nc.tensor.transpose(ps_t[:, :], s16[:, :], ident16[:, :])
                s_tr = smallp.tile([128, 128], fp16)
                nc.vector.tensor_copy(s_tr[:, :], ps_t[:, :])
                strow_d = dramp.tile([T, 128], fp16)
                nc.sync.dma_start(strow_d[:, :], s_tr[0:T, :])


            # ---- bias as a column (per-partition in transposed space) ------
            with tc.high_priority():
                # table pre-warm so the first real ACT op isn't stuck behind
                # the one-time activation-table load
                act_warm = smallp.tile([1, 1], f32)
                nc.scalar.activation(act_warm[:, :], m_bc[0:1, 0:1],
                                     mybir.ActivationFunctionType.Identity,
                                     bias=0.0, scale=1.0)
                btot = smallp.tile([1, F], f32)
                nc.vector.tensor_reduce(
                    btot[:, :], bkb_sb.rearrange("p (s f) -> p f s", s=KCH + 1),
                    axis=X, op=A.add)
                one1 = smallp.tile([1, 1], f32)
                nc.vector.memset(one1[:, :], 1.0)
                ps_bc = pst.tile([128, 1], f32, tag="pst")
                nc.tensor.matmul(ps_bc[:, :], btot[:, :], one1[:, :],
                                 start=True, stop=True)
                btot_col = smallp.tile([128, 1], f32)
                nc.vector.tensor_copy(btot_col[:, :], ps_bc[:, :])

            # ---- node features --------------------------------------------
            xt_sb = []
            for c in range(XCH):
                xt_c = xtp.tile([128, CW], mmdt, name=f"xt{c}")
                nc.sync.dma_start(xt_c[:, :], xt_d[:, c * CW : (c + 1) * CW])
                xt_sb.append(xt_c)

            # ---- main loop -------------------------------------------------
            sflat = strow_d[:, :]

            for c in range(XCH):
                out_c = outp.tile([128, CW], odt, name=f"out{c}")
                # one broadcast DMA per chunk: every partition re-reads this
                # chunk's node-ordered s row from DRAM
                srep_c = srepp.tile([128, CW], fp16, tag="srep")
                srcap = bass.AP(sflat.tensor, sflat.offset + c * CW, [[0, 128], [1, CW]])
                nc.sync.dma_start(srep_c[:, :], srcap)
                for gp in range(GPC // 2):
                    # paired groups: stationary sequence wa,wa,wb,wb
                    n0s = [(2 * gp + i) * GW for i in range(2)]
                    sxs, psFs = [], []
                    for n0 in n0s:
                        sx = sxp.tile([128, GW], mmdt, tag="sx")
                        nc.vector.tensor_tensor(sx[:, :], xt_sb[c][:, n0 : n0 + GW],
                                                srep_c[:, n0 : n0 + GW], op=A.mult)
                        sxs.append(sx)
                    for n0 in n0s:
                        psF = psf.tile([128, GW], f32, tag="psf")
                        nc.tensor.matmul(psF[:, :], wa16[:, :],
                                         xt_sb[c][:, n0 : n0 + GW], start=True, stop=False)
                        psFs.append(psF)
                    for n0, sx, psF in zip(n0s, sxs, psFs):
                        nc.tensor.matmul(psF[:, :], wb16[:, :], sx[:, :],
                                         start=False, stop=True)
                    for n0, psF in zip(n0s, psFs):
                        nc.scalar.activation(out_c[:, n0 : n0 + GW], psF[:, :],
                                             mybir.ActivationFunctionType.Identity,
                                             bias=btot_col[:, 0:1], scale=1.0)
                nc.sync.dma_start(out_d[:, c * CW : (c + 1) * CW], out_c[:, :])

    nc.compile()
    return nc


def _ceil8(x):
    return max(DSLOT_MIN, (int(x) + 7) // 8 * 8)


def _route_edges(vals, idx, dslot):
    """Host-side edge routing: permutation + zero-fill only (layout for the
    device segment-sum; no float arithmetic happens here)."""
    order = np.argsort(idx, kind="stable")
    si = idx[order]
    sv = vals[order]
    cnt = np.bincount(idx, minlength=N_NODES)
    first = np.concatenate(([0], np.cumsum(cnt)[:-1]))
    slot = np.arange(idx.shape[0], dtype=np.int64) - first[si]
    core = si // NPC
    ln = si - core * NPC
    rows = ln % 128
    cols = (ln // 128) * dslot + slot
    packed = np.zeros((NCORES, 128, T * dslot), np.float16)
    packed[core, rows, cols] = sv
    return packed


def kernel(nodes, edges, senders, receivers, Wk, bk, bias):
    nodes = np.ascontiguousarray(np.asarray(nodes, np.float32))
    edges = np.ascontiguousarray(np.asarray(edges, np.float32))
    senders = np.asarray(senders)
    receivers = np.asarray(receivers)
    Wk = np.ascontiguousarray(np.asarray(Wk, np.float32))
    bk = np.asarray(bk, np.float32)
    bias = np.asarray(bias, np.float32)
    assert nodes.shape == (N_NODES, F) and Wk.shape == (KCH, F, F)

    ds = _ceil8(np.bincount(senders, minlength=N_NODES).max())
    dr = _ceil8(np.bincount(receivers, minlength=N_NODES).max())

    if ("edge", ds, dr) not in _prog_cache:
        _prog_cache[("edge", ds, dr)] = _build_edge_program(ds, dr)
    if ("main", MM_MODE) not in _prog_cache:
        if MM_MODE.startswith("t"):
            _prog_cache[("main", MM_MODE)] = _build_main_program_t(MM_MODE)
        else:
            _prog_cache[("main", MM_MODE)] = _build_main_program(MM_MODE)
    ncA = _prog_cache[("edge", ds, dr)]
    ncB = _prog_cache[("main", MM_MODE)]
    transposed = MM_MODE.startswith("t")

    pse = _route_edges(edges, senders, ds)
    pre = _route_edges(edges, receivers, dr)
    bkb = np.ascontiguousarray(
        np.concatenate([bk.reshape(1, -1), bias.reshape(1, -1)], axis=1), np.float32)

    cores = list(range(NCORES))
    in_a = [{"pse": np.ascontiguousarray(pse[c]),
             "pre": np.ascontiguousarray(pre[c])} for c in cores]
    res_a = run_bass_kernel_spmd(ncA, in_a, cores, trace=TRACE)

    # combine the 8 device-computed partial maxima (selection, no arithmetic)
    m = max(float(res_a.results[c]["pmax"][0, 0]) for c in cores)
    mmax = np.array([[m]], np.float32)

    xdt = np.float16 if MM_MODE == "tfp16" else np.float32
    in_b = []
    for c in cores:
        xt = np.zeros((F, NPAD), xdt)
        xt[:, :NPC] = nodes[c * NPC : (c + 1) * NPC].T
        in_b.append({
            "xt": xt,
            "wk": Wk,
            "bkb": bkb,
            "degw": res_a.results[c]["degw"],
            "mmax": mmax,
        })
    res_b = run_bass_kernel_spmd(ncB, in_b, cores, trace=TRACE)

    ta = res_a.exec_time_ns
    tb = res_b.exec_time_ns
    LAST["exec_a_ns"] = ta
    LAST["exec_b_ns"] = tb
    LAST["exec_time_ns"] = (ta + tb) if (ta is not None and tb is not None) else None

    out = np.empty((N_NODES, F), np.float32)
    for c in cores:
        o = res_b.results[c]["out"]
        if transposed:
            out[c * NPC : (c + 1) * NPC] = o.astype(np.float32).T[:NPC]
        else:
            out[c * NPC : (c + 1) * NPC] = o[:NPC]
    return out
